# revision 11
# baseline (speedup 1.0000x reference)
"""Trainium2 Bass kernel for nn_CacheAttention (retrieval KNN attention).

Reference computation (per token, fully independent across tokens):
    q = (hidden @ Wq.T) * D**-0.5          # [t, H*D] viewed [t, KV, G, D]
    k = retrieved @ Wk.T                   # [t, N, KV*D] viewed [t, KV, N, D]
    v = retrieved @ Wv.T                   # viewed [t, KV, N, D]
    s = einsum('kgd,knd->kgn', q_t, k_t);  a = softmax(s, -1)
    out_t = einsum('kgn,knd->kgd', a, v_t).reshape(H*D) @ Wo.T

Strategy: data-parallel over the 4096 (b, s) tokens across 8 NeuronCores
(512 tokens each).  The host pre-transposes + bf16-casts all operands so
every matmul contracts over the SBUF partition dim with cheap, contiguous
DMA loads.

Attention is computed TRANSPOSED and diagonal-restricted: for each 8-token
sub-block, scores^T[(t',slot)=128, (t,g)=32] come from one PE matmul with
the K tile as stationary (contraction = d) plus a rank-16 mask matmul
(-30 off the t'==t diagonal) accumulated into the same PSUM group.  exp runs
on ACT straight out of PSUM; the softmax denominator is an all-ones
stationary matmul (partition-dim reduction on PE, already broadcast across
partitions); DVE takes the reciprocal; Pool multiplies exp * recip into the
normalized A^T chunks that feed the A.V matmuls directly.  This removes the
baseline's A^T XBAR DMA-transposes (16MB/core), its DVE mask-add, and 4x of
the exp/score volume, at the cost of small extra PE matmuls.
"""

import os
import sys

import numpy as np
import ml_dtypes

for _p in ("/opt/trn_rl_repo", "/root/.axon_site/_ro/trn_rl_repo"):
    if os.path.isdir(_p) and _p not in sys.path:
        sys.path.insert(0, _p)

import concourse.bass as bass  # noqa: E402
import concourse.mybir as mybir  # noqa: E402
import concourse.tile as tile  # noqa: E402
from concourse import bacc  # noqa: E402
from concourse.bass_utils import run_bass_kernel_spmd  # noqa: E402

# Problem shapes (hardcoded per contest contract).
B, S, HID = 2, 2048, 4096
H, KV, D = 32, 8, 128
G = H // KV  # 4
N = 16
RH = HID // 4  # 1024
NCORES = 8
TOK = B * S  # 4096 tokens total
T = TOK // NCORES  # 512 tokens per core
TBLK = 32  # tokens per pipeline block
NBLK = T // TBLK  # 16
SUB = 4  # 8-token sub-blocks per block
TN = T * N  # 8192 (token, neighbor) rows per core
TNBLK = TBLK * N  # 512
KC = HID // 128  # 32 contraction chunks for Q/O projections
RC = RH // 128  # 8 contraction chunks for K/V projections
SCALE = float(D) ** -0.5
MASK_NEG = -30.0

BF16 = mybir.dt.bfloat16
F32 = mybir.dt.float32
EXP = mybir.ActivationFunctionType.Exp

_NC = None


def _build_program(reps=1):
    nc = bacc.Bacc(None, target_bir_lowering=False, debug=False)

    hT = nc.dram_tensor("hT", [HID, T], BF16, kind="ExternalInput")
    rT = nc.dram_tensor("rT", [RH, TN], BF16, kind="ExternalInput")
    wqT = nc.dram_tensor("wqT", [HID, H * D], BF16, kind="ExternalInput")
    wkT = nc.dram_tensor("wkT", [RH, KV * D], BF16, kind="ExternalInput")
    wvT = nc.dram_tensor("wvT", [RH, KV * D], BF16, kind="ExternalInput")
    woT = nc.dram_tensor("woT", [H * D, HID], BF16, kind="ExternalInput")
    ones = nc.dram_tensor("ones", [128, 128], BF16, kind="ExternalInput")
    maskL = nc.dram_tensor("maskL", [16, 128], BF16, kind="ExternalInput")
    maskR = nc.dram_tensor("maskR", [16, 128], BF16, kind="ExternalInput")
    out = nc.dram_tensor("out", [T, HID], F32, kind="ExternalOutput")

    hT_r = hT[:].rearrange("(c p) t -> p c t", p=128)
    rT_r = rT[:].rearrange("(c p) t -> p c t", p=128)
    wq_r = wqT[:].rearrange("(c p) m -> p c m", p=128)
    wk_r = wkT[:].rearrange("(c p) m -> p c m", p=128)
    wv_r = wvT[:].rearrange("(c p) m -> p c m", p=128)
    wo_r = woT[:].rearrange("(c p) h -> p c h", p=128)
    out_r = out[:].rearrange("(mt p) h -> p mt h", p=128)

    with tile.TileContext(nc) as tc:
      for _rep in range(reps):  # >1 only for timing calibration builds
        with (
            tc.tile_pool(name="resident", bufs=1) as resp,
            tc.tile_pool(name="rt", bufs=2) as rtp,
        ):
            ones_sb = resp.tile([128, 128], BF16)
            maskL_sb = resp.tile([16, 128], BF16)
            maskR_sb = resp.tile([16, 128], BF16)
            wk_sb = resp.tile([128, RC, KV * D], BF16)
            wv_sb = resp.tile([128, RC, KV * D], BF16)
            # Q^T layout [d, kv, (blk, t, g)]: the qk moving slice for an
            # 8-token sub-block s of block blk is then the contiguous
            # [128, 32] range at blk*128 + 32*s.
            qT_sb = resp.tile([128, KV, G * T], BF16)
            qT_w = qT_sb[:].rearrange("p h (a t g) -> p h a t g", t=32, g=G)
            aoT_sb = resp.tile([128, H * D // 128, T], BF16)

            # ---- Stage 1: Q^T[(kv,t,g... d), t] = WqT.T-chunks x hT ----
            with (
                tc.tile_pool(name="hpool", bufs=1) as hp,
                tc.tile_pool(name="wq", bufs=12) as wqp,
                tc.tile_pool(name="ps1", bufs=2, space="PSUM") as ps1,
            ):
                hT_sb = hp.tile([128, KC, T], BF16)
                rt0 = rtp.tile([128, RC, TNBLK], BF16, tag="rt")
                for ms in range(8):  # 512-col slabs of Wq^T
                    slabs = []
                    for kg in range(KC // 4):
                        # 4-chunk-batched transfers: ms=0 needs 64 tiles and
                        # is DMA-bound with per-transfer overheads; batching
                        # keeps stage-1 start compute-bound.
                        sl = wqp.tile([128, 4, 512], BF16, tag="wqslab")
                        nc.sync.dma_start(
                            sl[:], wq_r[:, 4 * kg : 4 * kg + 4, ms * 512 : (ms + 1) * 512]
                        )
                        if ms == 0:
                            nc.sync.dma_start(
                                hT_sb[:, 4 * kg : 4 * kg + 4, :],
                                hT_r[:, 4 * kg : 4 * kg + 4, :],
                            )
                        slabs.append(sl)
                    if ms == 1:
                        # Prefetch block 0's retrieved states + stage-2
                        # weights while Q-projection compute runs (after the
                        # DMA-bound ms=0 group so they don't delay its slabs).
                        nc.sync.dma_start(rt0[:], rT_r[:, :, 0:TNBLK])
                        nc.sync.dma_start(ones_sb[:], ones[:])
                        nc.sync.dma_start(maskL_sb[:], maskL[:])
                        nc.sync.dma_start(maskR_sb[:], maskR[:])
                    elif ms == 2:
                        nc.sync.dma_start(wk_sb[:], wk_r)
                        nc.sync.dma_start(wv_sb[:], wv_r)
                    for mi in range(4):
                        m = ms * 4 + mi
                        qps = ps1.tile([128, 512], F32, tag="qps")
                        for k in range(KC):
                            nc.tensor.matmul(
                                qps[:],
                                slabs[k // 4][:, k % 4, mi * 128 : (mi + 1) * 128],
                                hT_sb[:, k, :],
                                start=(k == 0),
                                stop=(k == KC - 1),
                            )
                        # Fold the D**-0.5 query scaling into the PSUM evict.
                        nc.scalar.mul(
                            qT_w[:, m // G, :, :, m % G],
                            qps[:].rearrange("p (a t) -> p a t", t=32),
                            SCALE,
                        )

            # ---- Stage 2: per 32-token block: K/V projections + attention ----
            # Software-pipelined two deep: block b's den/recip/normalize and
            # A.V matmuls are emitted inside block b+1's K/V projection
            # stream so the PE never waits on the ACT/DVE/Pool softmax chain.
            with (
                tc.tile_pool(name="kt", bufs=2) as ktp,
                tc.tile_pool(name="vt", bufs=2) as vtp,
                tc.tile_pool(name="et", bufs=16) as etp,
                tc.tile_pool(name="rec", bufs=8) as recp,
                tc.tile_pool(name="at", bufs=16) as atp,
                tc.tile_pool(name="ps2", bufs=2, space="PSUM") as ps2,
                tc.tile_pool(name="pss", bufs=2, space="PSUM") as pss,
                tc.tile_pool(name="psd", bufs=2, space="PSUM") as psd,
                tc.tile_pool(name="psav", bufs=2, space="PSUM") as psav,
            ):

                def qk_stage(blk, ktile, et_list):
                    # scores^T for the 4 diagonal 8-token windows.  The mask
                    # matmul goes FIRST with start=True over the full tile
                    # (one rank-16 matmul covers all 4 windows); the qk
                    # matmuls then accumulate into their 32-col slices.
                    for kv in range(KV):
                        sps = pss.tile([128, SUB, 32], F32, tag="sps")
                        nc.tensor.matmul(
                            sps[:],
                            maskL_sb[:],
                            maskR_sb[:],
                            start=True,
                            stop=False,
                        )
                        for s_ in range(SUB):
                            nc.tensor.matmul(
                                sps[:, s_, :],
                                ktile[:, kv, 128 * s_ : 128 * (s_ + 1)],
                                qT_sb[:, kv, blk * 128 + 32 * s_ : blk * 128 + 32 * s_ + 32],
                                start=False,
                                stop=(s_ == SUB - 1),
                            )
                        et = etp.tile([128, SUB, 32], BF16, tag="et")
                        nc.scalar.activation(et[:], sps[:], EXP)
                        et_list.append(et)

                def den_stage(state):
                    # den row-broadcast via all-ones stationary (one matmul
                    # sums each column over its own sub-block's partitions);
                    # reciprocal on DVE; normalized A^T chunks on Pool.
                    blk, ktile, vflat, et_list, at_list = state
                    for kv in range(KV):
                        dps = psd.tile([128, 128], F32, tag="dps")
                        nc.tensor.matmul(
                            dps[:],
                            ones_sb[:],
                            et_list[kv][:],
                            start=True,
                            stop=True,
                        )
                        rec = recp.tile([128, 128], F32, tag="rec")
                        nc.vector.reciprocal(rec[:], dps[:])
                        at = atp.tile([128, SUB, 32], BF16, tag="at")
                        nc.vector.tensor_mul(
                            at[:],
                            et_list[kv][:],
                            rec[:].rearrange("p (s j) -> p s j", j=32),
                        )
                        at_list.append(at)

                def av_stage(state):
                    blk, ktile, vflat, et_list, at_list = state
                    for kv in range(KV):
                        avps = psav.tile([128, 128], F32, tag="av")
                        for s_ in range(SUB):
                            nc.tensor.matmul(
                                avps[:, 32 * s_ : 32 * s_ + 32],
                                vflat[:, kv, s_, :],
                                at_list[kv][:, s_, :],
                                start=True,
                                stop=True,
                            )
                        nc.vector.tensor_copy(
                            aoT_sb[
                                :,
                                kv * G : (kv + 1) * G,
                                blk * 32 : blk * 32 + 32,
                            ],
                            avps[:].rearrange("p (t g) -> p g t", g=G),
                        )

                prev = None
                for blk in range(NBLK):
                    if blk == 0:
                        rtile = rt0
                    else:
                        rtile = rtp.tile([128, RC, TNBLK], BF16, tag="rt")
                        nc.sync.dma_start(
                            rtile[:], rT_r[:, :, blk * TNBLK : (blk + 1) * TNBLK]
                        )

                    # The reference's torch flat-view [t,n,KV*D] -> [t,KV,n,D]
                    # means head kv attends slot nn drawn from neighbor
                    # n_src = 2*kv + nn//8 with kv-slice kvc = nn%8.  Softmax
                    # is permutation-invariant per head, so we only need a
                    # consistent slot order for K and V: slot = (n_src%2)*8
                    # + kvc, gathered on the free dim during PSUM eviction.
                    # K^T[d, head, (t, slot)] for this block.
                    ktile = ktp.tile([128, KV, TNBLK], BF16, tag="kt")
                    kdst = ktile[:].rearrange("p h (t b e) -> p t h b e", b=2, e=8)
                    for kvc in range(KV):
                        kps = ps2.tile([128, 512], F32, tag="ps2")
                        for k in range(RC):
                            nc.tensor.matmul(
                                kps[:],
                                wk_sb[:, k, kvc * 128 : (kvc + 1) * 128],
                                rtile[:, k, :],
                                start=(k == 0),
                                stop=(k == RC - 1),
                            )
                        nc.scalar.copy(
                            kdst[:, :, :, :, kvc],
                            kps[:].rearrange("p (t a b) -> p t a b", a=8, b=2),
                        )

                    if prev is not None:
                        den_stage(prev)

                    # V^T[d, head, (t, slot)], same gather as K; then XBAR
                    # DMA-transpose per head to V_flat[(t,slot), d].
                    vht = vtp.tile([128, KV, TNBLK], BF16, tag="vht")
                    vdst = vht[:].rearrange("p h (t b e) -> p t h b e", b=2, e=8)
                    for kvc in range(KV):
                        vps = ps2.tile([128, 512], F32, tag="ps2")
                        for k in range(RC):
                            nc.tensor.matmul(
                                vps[:],
                                wv_sb[:, k, kvc * 128 : (kvc + 1) * 128],
                                rtile[:, k, :],
                                start=(k == 0),
                                stop=(k == RC - 1),
                            )
                        nc.vector.tensor_copy(
                            vdst[:, :, :, :, kvc],
                            vps[:].rearrange("p (t a b) -> p t a b", a=8, b=2),
                        )
                    vflat = vtp.tile([128, KV, TNBLK // 128, D], BF16, tag="vflat")
                    for kv in range(KV):
                        nc.sync.dma_start(
                            vflat[:, kv, :, :], vht[:, kv, :], transpose=True
                        )

                    if prev is not None:
                        av_stage(prev)

                    et_list = []
                    qk_stage(blk, ktile, et_list)
                    prev = (blk, ktile, vflat, et_list, [])

                den_stage(prev)
                av_stage(prev)

            # ---- Stage 3: out[t, hid] = attnout^T-chunks.T x WoT ----
            with (
                tc.tile_pool(name="wo", bufs=2) as wop,
                tc.tile_pool(name="osb", bufs=3) as osp,
                tc.tile_pool(name="ps3", bufs=2, space="PSUM") as ps3,
            ):
                for f in range(HID // 512):
                    wsl = wop.tile([128, KC, 512], BF16, tag="wo")
                    if f == 0:
                        # Sub-chunked so the first O-proj matmuls start
                        # after ~1.6us instead of a full-slab 12.6us wait.
                        for kc in range(8):
                            nc.sync.dma_start(
                                wsl[:, 4 * kc : 4 * kc + 4, :],
                                wo_r[:, 4 * kc : 4 * kc + 4, 0:512],
                            )
                    else:
                        nc.sync.dma_start(wsl[:], wo_r[:, :, f * 512 : (f + 1) * 512])
                    for m in range(T // 128):
                        ops_ = ps3.tile([128, 512], F32, tag="ps3")
                        for k in range(KC):
                            nc.tensor.matmul(
                                ops_[:],
                                aoT_sb[:, k, m * 128 : (m + 1) * 128],
                                wsl[:, k, :],
                                start=(k == 0),
                                stop=(k == KC - 1),
                            )
                        ob = osp.tile([128, 512], F32, tag="ob")
                        nc.scalar.copy(ob[:], ops_[:])
                        nc.sync.dma_start(out_r[:, m, f * 512 : (f + 1) * 512], ob[:])

    nc.compile()
    return nc


def _get_nc():
    global _NC
    if _NC is None:
        _NC = _build_program()
    return _NC


def _mask_mats():
    """Rank-16 factorization of the -30 off-diagonal mask, tiled for all 4
    8-token sub-blocks: mask[r, (s,j)] = -30 + 30*[r//16 == j//4]."""
    mL = np.zeros((16, 128), dtype=np.float32)
    mR = np.zeros((16, 128), dtype=np.float32)
    mL[0, :] = 1.0
    mR[0, :] = MASK_NEG
    j = np.arange(128) % 32
    for i in range(8):
        mL[1 + i, :] = (np.arange(128) // 16 == i).astype(np.float32)
        mR[1 + i, :] = -MASK_NEG * (j // 4 == i).astype(np.float32)
    return mL, mR


def build_in_maps(hidden_states, retrieved_hidden_states, Wq, Wk, Wv, Wo):
    """Host-side sharding: pre-transpose + bf16-cast, slice tokens per core."""
    bf = ml_dtypes.bfloat16
    h2 = np.asarray(hidden_states, dtype=np.float32).reshape(TOK, HID).astype(bf)
    r2 = (
        np.asarray(retrieved_hidden_states, dtype=np.float32)
        .reshape(TOK * N, RH)
        .astype(bf)
    )
    wqT = np.ascontiguousarray(np.asarray(Wq, dtype=np.float32).astype(bf).T)
    wkT = np.ascontiguousarray(np.asarray(Wk, dtype=np.float32).astype(bf).T)
    wvT = np.ascontiguousarray(np.asarray(Wv, dtype=np.float32).astype(bf).T)
    woT = np.ascontiguousarray(np.asarray(Wo, dtype=np.float32).astype(bf).T)
    ones = np.ones((128, 128), dtype=np.float32).astype(bf)
    mL, mR = _mask_mats()
    mL = mL.astype(bf)
    mR = mR.astype(bf)

    in_maps = []
    for i in range(NCORES):
        hT_i = np.ascontiguousarray(h2[i * T : (i + 1) * T].T)
        rT_i = np.ascontiguousarray(r2[i * TN : (i + 1) * TN].T)
        in_maps.append(
            {
                "hT": hT_i,
                "rT": rT_i,
                "wqT": wqT,
                "wkT": wkT,
                "wvT": wvT,
                "woT": woT,
                "ones": ones,
                "maskL": mL,
                "maskR": mR,
            }
        )
    return in_maps


def kernel(hidden_states, retrieved_hidden_states, Wq, Wk, Wv, Wo):
    nc = _get_nc()
    in_maps = build_in_maps(
        hidden_states, retrieved_hidden_states, Wq, Wk, Wv, Wo
    )
    res = run_bass_kernel_spmd(nc, in_maps, core_ids=list(range(NCORES)))
    outs = [res.results[i]["out"] for i in range(NCORES)]
    full = np.concatenate(outs, axis=0).reshape(B, S, HID)
    return full
